# revision 14
# baseline (speedup 1.0000x reference)
"""GNN message-passing (GraphConv x4 + mean readout + linear classifier) on 8 TRN2 cores.

Sharding: dst-node (and incident-edge) partitioning across 8 cores. Each layer:
  - every core holds the full node-feature table (fp16) in DRAM (replicated via AllGather)
  - per 128-dst block: dma_gather src rows (fp16), build per-tile selection matrices
    S^T[e,d] = w_e * (dst_local[e]==d) on DVE, aggregate m^T via TensorE into PSUM,
    then h' = relu(m @ W + b) and write the core's slice; AllGather -> next table.
Readout: per-block matmul against graph-selection weights (1/cnt folded in),
AllReduce, then classifier matmul.  Dominant traffic: 256B/edge/layer gather.
"""

import math
from contextlib import ExitStack
from dataclasses import dataclass, field

import numpy as np

P = 128  # partitions; also feature dim and max graph count here


# --------------------------------------------------------------------------
# Planning: pack edges into per-core, per-superblock, per-half tile slots.
#
# Flat slot order per core:
#   for each superblock sb (SB blocks):
#     [all half0 tiles: block b0..b_last, K0 tiles each]
#     [all half1 tiles: block b0..b_last, K1 tiles each]
# Each tile is 128 slots (one gathered edge row per partition).
# --------------------------------------------------------------------------

@dataclass
class Plan:
    N: int
    E: int
    D: int
    C: int
    G: int
    NC: int
    NPC: int
    NB: int
    K0: int
    K1: int
    SB: int
    HALF: int
    n_layers: int
    src16: list = field(default_factory=list)   # [P, slots//16] int16 (row-replicated x8)
    dl: list = field(default_factory=list)      # [P, ntiles] fp16 dst_local (128=pad)
    scc: list = field(default_factory=list)     # [P, NB] fp32 c_src*c_dst per node
    scd: list = field(default_factory=list)     # [P, NB] fp32 c_dst per node
    icd: list = field(default_factory=list)     # [1, NB*P] fp16 1/c_dst per node
    gidf: list = field(default_factory=list)    # [P, NB] fp32 graph id per node
    invc: list = field(default_factory=list)    # [P, NB] fp32 1/cnt per node

    @property
    def nsb(self):
        return math.ceil(self.NB / self.SB)

    def sb_blocks(self, sb):
        return range(sb * self.SB, min((sb + 1) * self.SB, self.NB))

    @property
    def ntiles(self):
        return self.NB * (self.K0 + self.K1)

    @property
    def slots(self):
        return self.ntiles * P

    def tile_index(self, b, t):
        """Global tile index for block b, per-block tile t (t<K0: half0)."""
        sb, bl = b // self.SB, b % self.SB
        nblk = len(self.sb_blocks(sb))
        base = sb * self.SB * (self.K0 + self.K1)  # tiles before this sb
        if t < self.K0:
            return base + bl * self.K0 + t
        return base + nblk * self.K0 + bl * self.K1 + (t - self.K0)


def make_plan(x, edge_index, graph_ids, n_layers=4, NC=8, SB=4, C=10, G=None):
    N, D = x.shape
    E = edge_index.shape[1]
    if G is None:
        G = int(np.asarray(graph_ids).max()) + 1
    assert G <= P and D == P
    src = np.asarray(edge_index[0], dtype=np.int64)
    dst = np.asarray(edge_index[1], dtype=np.int64)
    NPC = math.ceil(N / NC)
    NB = math.ceil(NPC / P)
    HALF = math.ceil(N / 2)
    assert HALF <= 32767 and N - HALF <= 32767

    out_deg = np.bincount(src, minlength=N).astype(np.float64)
    in_deg = np.bincount(dst, minlength=N).astype(np.float64)
    c_src = np.clip(out_deg, 1.0, None) ** -0.5
    c_dst = np.clip(in_deg, 1.0, None) ** -0.5
    w_all = (c_src[src] * c_dst[dst]).astype(np.float32)

    core_of_edge = dst // NPC
    blk_in_core = (dst - core_of_edge * NPC) // P
    half_of_edge = (src >= HALF).astype(np.int64)
    key = np.lexsort((src, half_of_edge, blk_in_core, core_of_edge))
    src_s, w_s = src[key], w_all[key]
    dst_s = dst[key]
    half_s = half_of_edge[key]
    core_s = core_of_edge[key]
    blk_s = blk_in_core[key]

    K0 = K1 = 1
    per_block = {}
    for c in range(NC):
        mc = core_s == c
        for b in range(NB):
            mb = mc & (blk_s == b)
            e0 = int((half_s[mb] == 0).sum())
            e1 = int(mb.sum()) - e0
            per_block[(c, b)] = mb
            K0 = max(K0, math.ceil(max(e0, 1) / P))
            K1 = max(K1, math.ceil(max(e1, 1) / P))

    plan = Plan(N=N, E=E, D=D, C=C, G=G, NC=NC, NPC=NPC, NB=NB, K0=K0, K1=K1,
                SB=SB, HALF=HALF, n_layers=n_layers)

    cnt = np.bincount(np.asarray(graph_ids, dtype=np.int64), minlength=G).astype(np.float64)
    invc_all = 1.0 / np.clip(cnt, 1.0, None)

    # per-node normalization folded into tables/activations:
    #   table_l = c_src * h_l ; h' = relu(cc * (m_raw @ W + invcd x b))
    plan.c_src, plan.c_dst = c_src.astype(np.float32), c_dst.astype(np.float32)
    ntiles = plan.ntiles
    for c in range(NC):
        src_flat = np.zeros(ntiles * P, dtype=np.int16)
        dl_flat = np.full(ntiles * P, 128.0, dtype=np.float16)  # 128 = no match
        for b in range(NB):
            mb = per_block[(c, b)]
            s_b, d_b, h_b = src_s[mb], dst_s[mb], half_s[mb]
            dloc = (d_b - c * NPC - b * P).astype(np.float16)
            for half, K in ((0, K0), (1, K1)):
                sel = h_b == half
                idx = s_b[sel] - (HALF if half else 0)
                t0 = plan.tile_index(b, 0 if half == 0 else K0)
                base = t0 * P
                n = len(idx)
                src_flat[base:base + n] = idx.astype(np.int16)
                dl_flat[base:base + n] = dloc[sel]

        wrapped = src_flat.reshape(-1, 16).T          # [16, slots/16]
        plan.src16.append(np.ascontiguousarray(np.tile(wrapped, (8, 1))))
        plan.dl.append(np.ascontiguousarray(dl_flat.reshape(-1, P).T))   # [P, ntiles]
        # per-block-node scale/bias-fold arrays
        lo, hi = c * NPC, min((c + 1) * NPC, N)
        cs = np.zeros(NB * P, np.float32); cs[:hi - lo] = c_src[lo:hi]
        cd = np.ones(NB * P, np.float32); cd[:hi - lo] = c_dst[lo:hi]
        plan.scc.append(np.ascontiguousarray((cs * cd).reshape(NB, P).T))  # [P,NB]
        plan.scd.append(np.ascontiguousarray(cd.reshape(NB, P).T))         # [P,NB]
        plan.icd.append(np.ascontiguousarray((1.0 / cd).reshape(1, NB * P).astype(np.float16)))

        gidf = np.zeros(NB * P, dtype=np.float32)
        invc = np.zeros(NB * P, dtype=np.float32)
        lo, hi = c * NPC, min((c + 1) * NPC, N)
        gids = np.asarray(graph_ids[lo:hi], dtype=np.int64)
        gidf[:hi - lo] = gids.astype(np.float32)
        invc[:hi - lo] = invc_all[gids].astype(np.float32)
        plan.gidf.append(np.ascontiguousarray(gidf.reshape(NB, P).T))    # [P, NB]
        plan.invc.append(np.ascontiguousarray(invc.reshape(NB, P).T))    # [P, NB]

    return plan


# --------------------------------------------------------------------------
# Golden numpy model of the exact device algorithm (fp16 gather/aggregation).
# --------------------------------------------------------------------------

def golden(plan: Plan, x, W_all, b_all, Wc, bc):
    f16, f32 = np.float16, np.float32
    htab = (x * plan.c_src[:, None]).astype(f16)
    NPC, NB, K0, K1, HALF = plan.NPC, plan.NB, plan.K0, plan.K1, plan.HALF
    KT = K0 + K1
    iota = np.arange(P, dtype=f16)[None, :]
    h4_blocks = [[None] * NB for _ in range(plan.NC)]
    for layer in range(plan.n_layers):
        W16 = W_all[layer].astype(f16)
        nxt = np.zeros((plan.N, plan.D), dtype=f16)
        for c in range(plan.NC):
            flat_idx = plan.src16[c][:16, :].T.reshape(-1)
            dl = plan.dl[c]
            for b in range(NB):
                mT = np.zeros((plan.D, P), dtype=f32)
                for t in range(KT):
                    ti = plan.tile_index(b, t)
                    sl = flat_idx[ti * P:(ti + 1) * P].astype(np.int64)
                    base = 0 if t < K0 else HALF
                    M = htab[base + sl, :]
                    ST = (iota == dl[:, ti:ti + 1]).astype(f16)
                    mT += M.astype(f32).T @ ST.astype(f32)
                mT16 = mT.astype(f16)
                pre = mT16.astype(f32).T @ W16.astype(f32)
                icd = plan.icd[c][0, b * P:(b + 1) * P].astype(f32)
                pre += icd[:, None] @ b_all[layer].astype(f32)[None, :]
                scl = (plan.scc[c] if layer < plan.n_layers - 1 else
                       plan.scd[c])[:, b]
                hb = np.maximum(pre * scl[:, None], 0).astype(f16)
                h4_blocks[c][b] = hb
                lo = c * NPC + b * P
                hi = min(lo + P, min((c + 1) * NPC, plan.N))
                if hi > lo:
                    nxt[lo:hi] = hb[:hi - lo]
        htab = nxt
    pgT = np.zeros((plan.D, P), dtype=f32)
    for c in range(plan.NC):
        for b in range(NB):
            hb = h4_blocks[c][b]
            SgT = ((iota.astype(np.float32) == plan.gidf[c][:, b:b + 1]) * plan.invc[c][:, b:b + 1]).astype(f16)
            pgT += hb.astype(f32).T @ SgT.astype(f32)
    out = pgT.T @ Wc.astype(f32) + bc[None, :]
    return out[:plan.G].astype(f32)


# --------------------------------------------------------------------------
# Bass/Tile kernel builder.
# --------------------------------------------------------------------------

def build_inputs(plan: Plan, x, W_all, b_all, Wc, bc):
    """Per-core in_maps for run_bass_kernel_spmd."""
    iota = np.tile(np.arange(P, dtype=np.float16)[None, :], (P, 1))
    common = {
        "x16": np.ascontiguousarray((x * plan.c_src[:, None]).astype(np.float16)),
        "w16": np.ascontiguousarray(W_all.astype(np.float16)),
        "b16": np.ascontiguousarray(b_all.astype(np.float16).reshape(1, -1)),
        "wc32": np.ascontiguousarray(Wc.astype(np.float32)),
        "bc32": np.ascontiguousarray(bc.astype(np.float32).reshape(1, -1)),
        "iota16": iota,
    }
    maps = []
    for c in range(plan.NC):
        m = dict(common)
        m["src16"] = plan.src16[c]
        m["dl16"] = plan.dl[c]
        m["scc32"] = plan.scc[c]
        m["scd32"] = plan.scd[c]
        m["icd16"] = plan.icd[c]
        m["gid16"] = plan.gidf[c]
        m["ivc16"] = plan.invc[c]
        maps.append(m)
    return maps


def build_nc(plan: Plan, num_swdge_queues=4, use_collectives=True):
    import concourse.bass as bass
    import concourse.tile as tile
    from concourse import bacc, mybir
    from concourse.tile_rust import add_dep_helper

    def _inst(i):
        return i.ins if hasattr(i, "ins") and not hasattr(i, "engine") else i

    f16, f32, i16 = mybir.dt.float16, mybir.dt.float32, mybir.dt.int16
    NB, K0, K1, SB = plan.NB, plan.K0, plan.K1, plan.SB
    KT = K0 + K1
    NL = plan.n_layers

    nc = bacc.Bacc(
        "TRN2",
        target_bir_lowering=False,
        debug=False,
        num_devices=plan.NC,
        num_swdge_queues=num_swdge_queues,
        dynamic_dma_scratch_size=49152,
    )
    rg = [list(range(plan.NC))]

    # ---- DRAM I/O ----
    x16 = nc.dram_tensor("x16", [plan.N, P], f16, kind="ExternalInput")
    w16 = nc.dram_tensor("w16", [NL, P, P], f16, kind="ExternalInput")
    b16 = nc.dram_tensor("b16", [1, NL * P], f16, kind="ExternalInput")
    wc32 = nc.dram_tensor("wc32", [P, plan.C], f32, kind="ExternalInput")
    bc32 = nc.dram_tensor("bc32", [1, plan.C], f32, kind="ExternalInput")
    iota16 = nc.dram_tensor("iota16", [P, P], f16, kind="ExternalInput")
    src16 = nc.dram_tensor("src16", [P, plan.slots // 16], i16, kind="ExternalInput")
    dl16 = nc.dram_tensor("dl16", [P, plan.ntiles], f16, kind="ExternalInput")
    scc32 = nc.dram_tensor("scc32", [P, NB], f32, kind="ExternalInput")
    scd32 = nc.dram_tensor("scd32", [P, NB], f32, kind="ExternalInput")
    icd16 = nc.dram_tensor("icd16", [1, NB * P], f16, kind="ExternalInput")
    gid16 = nc.dram_tensor("gid16", [P, NB], f32, kind="ExternalInput")
    ivc16 = nc.dram_tensor("ivc16", [P, NB], f32, kind="ExternalInput")
    out_d = nc.dram_tensor("out", [plan.G, plan.C], f32, kind="ExternalOutput")

    # internal DRAM: per-layer local slice + gathered full table
    hloc = [nc.dram_tensor(f"hloc{l}", [plan.NPC, P], f16) for l in range(NL - 1)]
    hfull = [nc.dram_tensor(f"hfull{l}", [plan.NPC * plan.NC, P], f16,
                            addr_space="Shared") for l in range(NL - 1)]
    pg_in = nc.dram_tensor("pg_in", [P, P], f32)
    pg_out = nc.dram_tensor("pg_out", [P, P], f32, addr_space="Shared")

    with tile.TileContext(nc) as tc, ExitStack() as ctx:
        const = ctx.enter_context(tc.tile_pool(name="const", bufs=1))
        gpool = ctx.enter_context(tc.tile_pool(name="gather", bufs=3))
        spool = ctx.enter_context(tc.tile_pool(name="sel", bufs=6))
        mpool = ctx.enter_context(tc.tile_pool(name="mt", bufs=3))
        hpool = ctx.enter_context(tc.tile_pool(name="hb", bufs=4))
        h4pool = ctx.enter_context(tc.tile_pool(name="h4", bufs=NB))
        psum_m = ctx.enter_context(tc.tile_pool(name="psum_m", bufs=2, space="PSUM"))
        psum_h = ctx.enter_context(tc.tile_pool(name="psum_h", bufs=2, space="PSUM"))
        psum_g = ctx.enter_context(tc.tile_pool(name="psum_g", bufs=1, space="PSUM"))
        opool = ctx.enter_context(tc.tile_pool(name="outp", bufs=1))

        # ---- constants into SBUF ----
        def cload(tag, dram, shape, dt):
            t = const.tile(shape, dt, tag=tag)
            nc.sync.dma_start(out=t[:], in_=dram[:])
            return t

        IOTA = cload("iota", iota16, [P, P], f16)
        SRC = cload("src", src16, [P, plan.slots // 16], i16)
        DL = cload("dl", dl16, [P, plan.ntiles], f16)
        SCC = cload("scc", scc32, [P, NB], f32)
        SCD = cload("scd", scd32, [P, NB], f32)
        ICD = cload("icd", icd16, [1, NB * P], f16)
        GID = cload("gid", gid16, [P, NB], f32)
        IVC = cload("ivc", ivc16, [P, NB], f32)
        WTS = [cload(f"wt{l}", w16[l], [P, P], f16) for l in range(NL)]
        BROW = cload("brow", b16, [1, NL * P], f16)
        WC = cload("wc", wc32, [P, plan.C], f32)
        BC = cload("bc", bc32, [1, plan.C], f32)
        ONE16 = const.tile([1, P], f16, tag="one16")
        nc.vector.memset(ONE16[:], 1.0)
        ONE32 = const.tile([1, P], f32, tag="one32")
        nc.vector.memset(ONE32[:], 1.0)

        is_eq = mybir.AluOpType.is_equal
        mult = mybir.AluOpType.mult

        def sel_tile(col_src, col):
            """Sg^T[n,g] = (iota_g == gid[n]) * invc[n], fp16 (readout only)."""
            st = spool.tile([P, P], f16, tag="sel")
            nc.vector.tensor_scalar(
                out=st[:], in0=IOTA[:],
                scalar1=col_src[0][:, col:col + 1],
                scalar2=col_src[1][:, col:col + 1],
                op0=is_eq, op1=mult)
            return st

        def sel_batch(t0, nt):
            """Binary S^T for nt consecutive tiles: [P, nt, P] fp16."""
            st = spool.tile([P, 9, P], f16, tag="selb")
            nc.vector.tensor_tensor(
                out=st[:, :nt, :],
                in0=IOTA[:].unsqueeze(1).to_broadcast([P, nt, P]),
                in1=DL[:, t0:t0 + nt].unsqueeze(2).to_broadcast([P, nt, P]),
                op=is_eq)
            return st

        prev_cc = None  # collective inst whose output feeds this layer's gathers
        h4_tiles = [None] * NB
        for layer in range(NL):
            table = x16 if layer == 0 else hfull[layer - 1]
            with nc.named_scope(f"conv{layer}"):
                for sb in range(plan.nsb):
                    blocks = list(plan.sb_blocks(sb))
                    nblk = len(blocks)
                    sbt = nblk * KT  # tiles in this superblock
                    g = gpool.tile([P, SB * KT, P], f16, tag="g")
                    t0 = plan.tile_index(blocks[0], 0)
                    # two gathers: half0 tiles then half1 tiles
                    for half, Kh, ntile_h, toff in (
                        (0, K0, nblk * K0, 0),
                        (1, K1, nblk * K1, nblk * K0),
                    ):
                        n_idx = ntile_h * P
                        col0 = (t0 + toff) * P // 16
                        tab = table[0:plan.HALF, :] if half == 0 else \
                            table[plan.HALF:plan.N, :]
                        gi = nc.gpsimd.dma_gather(
                            g[:, toff:toff + ntile_h, :],
                            tab,
                            SRC[:, col0:col0 + n_idx // 16],
                            n_idx, n_idx, P,
                            queue_num=sb % num_swdge_queues,
                            single_packet=False,
                        )
                        if prev_cc is not None:
                            add_dep_helper(_inst(gi), _inst(prev_cc), reason="gather after AG")

                    for bl, b in enumerate(blocks):
                        pm = psum_m.tile([P, P], f32, tag="pm")
                        st0 = sel_batch(plan.tile_index(b, 0), K0)
                        st1 = sel_batch(plan.tile_index(b, K0), K1)
                        for t in range(KT):
                            st = st0[:, t, :] if t < K0 else st1[:, t - K0, :]
                            gt = (bl * K0 + t) if t < K0 else \
                                (nblk * K0 + bl * K1 + (t - K0))
                            nc.tensor.matmul(
                                out=pm[:], lhsT=g[:, gt, :], rhs=st,
                                start=(t == 0), stop=(t == KT - 1))
                        mt = mpool.tile([P, P], f16, tag="mt")
                        nc.vector.tensor_copy(out=mt[:], in_=pm[:])
                        ph = psum_h.tile([P, P], f32, tag="ph")
                        nc.tensor.matmul(out=ph[:], lhsT=mt[:], rhs=WTS[layer][:],
                                         start=True, stop=False)
                        nc.tensor.matmul(out=ph[:],
                                         lhsT=ICD[0:1, b * P:(b + 1) * P],
                                         rhs=BROW[0:1, layer * P:(layer + 1) * P],
                                         start=False, stop=True)
                        if layer < NL - 1:
                            hb = hpool.tile([P, P], f16, tag="hb")
                        else:
                            hb = h4pool.tile([P, P], f16, tag="h4")
                        scl = SCC if layer < NL - 1 else SCD
                        nc.scalar.activation(
                            out=hb[:], in_=ph[:],
                            func=mybir.ActivationFunctionType.Relu,
                            scale=scl[:, b:b + 1])
                        if layer < NL - 1:
                            rows = min(plan.NPC - b * P, P)
                            nc.sync.dma_start(
                                out=hloc[layer][b * P:b * P + rows, :],
                                in_=hb[:rows, :])
                        else:
                            h4_tiles[b] = hb
            if layer < NL - 1:
                if use_collectives:
                    prev_cc = nc.gpsimd.collective_compute(
                        "AllGather", mybir.AluOpType.bypass,
                        ins=[hloc[layer].ap().opt()], outs=[hfull[layer].ap().opt()],
                        replica_groups=rg)
                else:
                    assert plan.NC == 1
                    prev_cc = nc.sync.dma_start(out=hfull[layer][:, :],
                                                in_=hloc[layer][:, :])

        # ---- readout ----
        with nc.named_scope("readout"):
            pg = psum_g.tile([P, P], f32, tag="pg")
            for b in range(NB):
                sg = sel_tile((GID, IVC), b)
                nc.tensor.matmul(out=pg[:], lhsT=h4_tiles[b][:], rhs=sg[:],
                                 start=(b == 0), stop=(b == NB - 1))
            pgs = opool.tile([P, P], f32, tag="pgs")
            nc.vector.tensor_copy(out=pgs[:], in_=pg[:])
            wr = nc.sync.dma_start(out=pg_in[:, :], in_=pgs[:])
            if use_collectives:
                cc = nc.gpsimd.collective_compute(
                    "AllReduce", mybir.AluOpType.add,
                    ins=[pg_in.ap().opt()], outs=[pg_out.ap().opt()],
                    replica_groups=rg)
            else:
                cc = nc.sync.dma_start(out=pg_out[:, :], in_=pg_in[:, :])
            hgT = opool.tile([P, P], f32, tag="hgT")
            rd = nc.sync.dma_start(out=hgT[:], in_=pg_out[:, :])
            add_dep_helper(_inst(rd), _inst(cc), reason="read after AR")
            po = psum_g.tile([P, plan.C], f32, tag="po")
            nc.tensor.matmul(out=po[:plan.G, :], lhsT=hgT[:, :plan.G], rhs=WC[:],
                             start=True, stop=False)
            nc.tensor.matmul(out=po[:plan.G, :], lhsT=ONE32[0:1, :plan.G], rhs=BC[:],
                             start=False, stop=True)
            ob = opool.tile([P, plan.C], f32, tag="ob")
            nc.vector.tensor_copy(out=ob[:plan.G, :], in_=po[:plan.G, :])
            nc.sync.dma_start(out=out_d[:, :], in_=ob[:plan.G, :])

    nc.compile()
    return nc


# --------------------------------------------------------------------------
# Entry point.
# --------------------------------------------------------------------------

_CACHE = {}


def _get_compiled(plan_key, plan):
    if plan_key not in _CACHE:
        _CACHE[plan_key] = build_nc(plan)
    return _CACHE[plan_key]


def kernel(x, W0, b0, Ws, bs, Wc, bc, edge_index, graph_ids):
    x = np.asarray(x)
    edge_index = np.asarray(edge_index)
    graph_ids = np.asarray(graph_ids)
    W_all = np.concatenate([np.asarray(W0)[None], np.asarray(Ws)], axis=0)
    b_all = np.concatenate([np.asarray(b0)[None], np.asarray(bs)], axis=0)
    Wc, bc = np.asarray(Wc), np.asarray(bc)

    plan = make_plan(x, edge_index, graph_ids)
    key = (plan.N, plan.E, plan.G, plan.K0, plan.K1, plan.n_layers)
    nc = _get_compiled(key, plan)

    from concourse.bass_utils import run_bass_kernel_spmd
    in_maps = build_inputs(plan, x, W_all, b_all, Wc, bc)
    res = run_bass_kernel_spmd(nc, in_maps, core_ids=list(range(plan.NC)))
    return res.results[0]["out"].astype(np.float32)


# revision 15
# speedup vs baseline: 1.4152x; 1.4152x over previous
"""GNN message-passing (GraphConv x4 + mean readout + linear classifier) on 8 TRN2 cores.

Sharding: dst-node (and incident-edge) partitioning across 8 cores. Each layer:
  - every core holds the full node-feature table (fp16) in DRAM (replicated via AllGather)
  - per 128-dst block: dma_gather src rows (fp16), build per-tile selection matrices
    S^T[e,d] = w_e * (dst_local[e]==d) on DVE, aggregate m^T via TensorE into PSUM,
    then h' = relu(m @ W + b) and write the core's slice; AllGather -> next table.
Readout: per-block matmul against graph-selection weights (1/cnt folded in),
AllReduce, then classifier matmul.  Dominant traffic: 256B/edge/layer gather.
"""

import math
from contextlib import ExitStack
from dataclasses import dataclass, field

import numpy as np

P = 128  # partitions; also feature dim and max graph count here


# --------------------------------------------------------------------------
# Planning: pack edges into per-core, per-superblock, per-half tile slots.
#
# Flat slot order per core:
#   for each superblock sb (SB blocks):
#     [all half0 tiles: block b0..b_last, K0 tiles each]
#     [all half1 tiles: block b0..b_last, K1 tiles each]
# Each tile is 128 slots (one gathered edge row per partition).
# --------------------------------------------------------------------------

@dataclass
class Plan:
    N: int
    E: int
    D: int
    C: int
    G: int
    NC: int
    NPC: int
    NB: int
    K0: int
    K1: int
    SB: int
    HALF: int
    n_layers: int
    src16: list = field(default_factory=list)   # [P, slots//16] int16 (row-replicated x8)
    dl: list = field(default_factory=list)      # [P, ntiles] fp16 dst_local (128=pad)
    scc: list = field(default_factory=list)     # [P, NB] fp32 c_src*c_dst per node
    scd: list = field(default_factory=list)     # [P, NB] fp32 c_dst per node
    icd: list = field(default_factory=list)     # [1, NB*P] fp16 1/c_dst per node
    gidf: list = field(default_factory=list)    # [P, NB] fp32 graph id per node
    invc: list = field(default_factory=list)    # [P, NB] fp32 1/cnt per node

    @property
    def nsb(self):
        return math.ceil(self.NB / self.SB)

    def sb_blocks(self, sb):
        return range(sb * self.SB, min((sb + 1) * self.SB, self.NB))

    @property
    def ntiles(self):
        return self.NB * (self.K0 + self.K1)

    @property
    def slots(self):
        return self.ntiles * P

    def tile_index(self, b, t):
        """Global tile index for block b, per-block tile t (t<K0: half0)."""
        sb, bl = b // self.SB, b % self.SB
        nblk = len(self.sb_blocks(sb))
        base = sb * self.SB * (self.K0 + self.K1)  # tiles before this sb
        if t < self.K0:
            return base + bl * self.K0 + t
        return base + nblk * self.K0 + bl * self.K1 + (t - self.K0)


def make_plan(x, edge_index, graph_ids, n_layers=4, NC=8, SB=4, C=10, G=None):
    N, D = x.shape
    E = edge_index.shape[1]
    if G is None:
        G = int(np.asarray(graph_ids).max()) + 1
    assert G <= P and D == P
    src = np.asarray(edge_index[0], dtype=np.int64)
    dst = np.asarray(edge_index[1], dtype=np.int64)
    NPC = math.ceil(N / NC)
    NB = math.ceil(NPC / P)
    HALF = math.ceil(N / 2)
    assert HALF <= 32767 and N - HALF <= 32767

    out_deg = np.bincount(src, minlength=N).astype(np.float64)
    in_deg = np.bincount(dst, minlength=N).astype(np.float64)
    c_src = np.clip(out_deg, 1.0, None) ** -0.5
    c_dst = np.clip(in_deg, 1.0, None) ** -0.5
    w_all = (c_src[src] * c_dst[dst]).astype(np.float32)

    core_of_edge = dst // NPC
    blk_in_core = (dst - core_of_edge * NPC) // P
    half_of_edge = (src >= HALF).astype(np.int64)
    key = np.lexsort((src, half_of_edge, blk_in_core, core_of_edge))
    src_s, w_s = src[key], w_all[key]
    dst_s = dst[key]
    half_s = half_of_edge[key]
    core_s = core_of_edge[key]
    blk_s = blk_in_core[key]

    K0 = K1 = 1
    per_block = {}
    for c in range(NC):
        mc = core_s == c
        for b in range(NB):
            mb = mc & (blk_s == b)
            e0 = int((half_s[mb] == 0).sum())
            e1 = int(mb.sum()) - e0
            per_block[(c, b)] = mb
            K0 = max(K0, math.ceil(max(e0, 1) / P))
            K1 = max(K1, math.ceil(max(e1, 1) / P))

    plan = Plan(N=N, E=E, D=D, C=C, G=G, NC=NC, NPC=NPC, NB=NB, K0=K0, K1=K1,
                SB=SB, HALF=HALF, n_layers=n_layers)

    cnt = np.bincount(np.asarray(graph_ids, dtype=np.int64), minlength=G).astype(np.float64)
    invc_all = 1.0 / np.clip(cnt, 1.0, None)

    # per-node normalization folded into tables/activations:
    #   table_l = c_src * h_l ; h' = relu(cc * (m_raw @ W + invcd x b))
    plan.c_src, plan.c_dst = c_src.astype(np.float32), c_dst.astype(np.float32)
    ntiles = plan.ntiles
    for c in range(NC):
        src_flat = np.zeros(ntiles * P, dtype=np.int16)
        dl_flat = np.full(ntiles * P, 128.0, dtype=np.float16)  # 128 = no match
        for b in range(NB):
            mb = per_block[(c, b)]
            s_b, d_b, h_b = src_s[mb], dst_s[mb], half_s[mb]
            dloc = (d_b - c * NPC - b * P).astype(np.float16)
            for half, K in ((0, K0), (1, K1)):
                sel = h_b == half
                idx = s_b[sel] - (HALF if half else 0)
                t0 = plan.tile_index(b, 0 if half == 0 else K0)
                base = t0 * P
                n = len(idx)
                src_flat[base:base + n] = idx.astype(np.int16)
                dl_flat[base:base + n] = dloc[sel]

        wrapped = src_flat.reshape(-1, 16).T          # [16, slots/16]
        plan.src16.append(np.ascontiguousarray(np.tile(wrapped, (8, 1))))
        plan.dl.append(np.ascontiguousarray(dl_flat.reshape(-1, P).T))   # [P, ntiles]
        # per-block-node scale/bias-fold arrays
        lo, hi = c * NPC, min((c + 1) * NPC, N)
        cs = np.zeros(NB * P, np.float32); cs[:hi - lo] = c_src[lo:hi]
        cd = np.ones(NB * P, np.float32); cd[:hi - lo] = c_dst[lo:hi]
        plan.scc.append(np.ascontiguousarray((cs * cd).reshape(NB, P).T))  # [P,NB]
        plan.scd.append(np.ascontiguousarray(cd.reshape(NB, P).T))         # [P,NB]
        plan.icd.append(np.ascontiguousarray((1.0 / cd).reshape(1, NB * P).astype(np.float16)))

        gidf = np.zeros(NB * P, dtype=np.float32)
        invc = np.zeros(NB * P, dtype=np.float32)
        lo, hi = c * NPC, min((c + 1) * NPC, N)
        gids = np.asarray(graph_ids[lo:hi], dtype=np.int64)
        gidf[:hi - lo] = gids.astype(np.float32)
        invc[:hi - lo] = invc_all[gids].astype(np.float32)
        plan.gidf.append(np.ascontiguousarray(gidf.reshape(NB, P).T))    # [P, NB]
        plan.invc.append(np.ascontiguousarray(invc.reshape(NB, P).T))    # [P, NB]

    return plan


# --------------------------------------------------------------------------
# Golden numpy model of the exact device algorithm (fp16 gather/aggregation).
# --------------------------------------------------------------------------

def golden(plan: Plan, x, W_all, b_all, Wc, bc):
    f16, f32 = np.float16, np.float32
    htab = (x * plan.c_src[:, None]).astype(f16)
    NPC, NB, K0, K1, HALF = plan.NPC, plan.NB, plan.K0, plan.K1, plan.HALF
    KT = K0 + K1
    iota = np.arange(P, dtype=f16)[None, :]
    h4_blocks = [[None] * NB for _ in range(plan.NC)]
    for layer in range(plan.n_layers):
        W16 = W_all[layer].astype(f16)
        nxt = np.zeros((plan.N, plan.D), dtype=f16)
        for c in range(plan.NC):
            flat_idx = plan.src16[c][:16, :].T.reshape(-1)
            dl = plan.dl[c]
            for b in range(NB):
                mT = np.zeros((plan.D, P), dtype=f32)
                for t in range(KT):
                    ti = plan.tile_index(b, t)
                    sl = flat_idx[ti * P:(ti + 1) * P].astype(np.int64)
                    base = 0 if t < K0 else HALF
                    M = htab[base + sl, :]
                    ST = (iota == dl[:, ti:ti + 1]).astype(f16)
                    mT += M.astype(f32).T @ ST.astype(f32)
                mT16 = mT.astype(f16)
                pre = mT16.astype(f32).T @ W16.astype(f32)
                icd = plan.icd[c][0, b * P:(b + 1) * P].astype(f32)
                pre += icd[:, None] @ b_all[layer].astype(f32)[None, :]
                scl = (plan.scc[c] if layer < plan.n_layers - 1 else
                       plan.scd[c])[:, b]
                hb = np.maximum(pre * scl[:, None], 0).astype(f16)
                h4_blocks[c][b] = hb
                lo = c * NPC + b * P
                hi = min(lo + P, min((c + 1) * NPC, plan.N))
                if hi > lo:
                    nxt[lo:hi] = hb[:hi - lo]
        htab = nxt
    pgT = np.zeros((plan.D, P), dtype=f32)
    for c in range(plan.NC):
        for b in range(NB):
            hb = h4_blocks[c][b]
            SgT = ((iota.astype(np.float32) == plan.gidf[c][:, b:b + 1]) * plan.invc[c][:, b:b + 1]).astype(f16)
            pgT += hb.astype(f32).T @ SgT.astype(f32)
    out = pgT.T @ Wc.astype(f32) + bc[None, :]
    return out[:plan.G].astype(f32)


# --------------------------------------------------------------------------
# Bass/Tile kernel builder.
# --------------------------------------------------------------------------

def build_inputs(plan: Plan, x, W_all, b_all, Wc, bc):
    """Per-core in_maps for run_bass_kernel_spmd."""
    iota = np.tile(np.arange(P, dtype=np.float16)[None, :], (P, 1))
    common = {
        "x16": np.ascontiguousarray((x * plan.c_src[:, None]).astype(np.float16)),
        "w16": np.ascontiguousarray(W_all.astype(np.float16)),
        "b16": np.ascontiguousarray(b_all.astype(np.float16).reshape(1, -1)),
        "wc32": np.ascontiguousarray(Wc.astype(np.float32)),
        "bc32": np.ascontiguousarray(bc.astype(np.float32).reshape(1, -1)),
        "iota16": iota,
    }
    maps = []
    for c in range(plan.NC):
        m = dict(common)
        m["src16"] = plan.src16[c]
        m["dl16"] = plan.dl[c]
        m["scc32"] = plan.scc[c]
        m["scd32"] = plan.scd[c]
        m["icd16"] = plan.icd[c]
        m["gid16"] = plan.gidf[c]
        m["ivc16"] = plan.invc[c]
        maps.append(m)
    return maps


def build_nc(plan: Plan, num_swdge_queues=4, use_collectives=True):
    import concourse.bass as bass
    import concourse.tile as tile
    from concourse import bacc, mybir
    from concourse.tile_rust import add_dep_helper

    def _inst(i):
        return i.ins if hasattr(i, "ins") and not hasattr(i, "engine") else i

    f16, f32, i16 = mybir.dt.float16, mybir.dt.float32, mybir.dt.int16
    NB, K0, K1, SB = plan.NB, plan.K0, plan.K1, plan.SB
    KT = K0 + K1
    NL = plan.n_layers

    nc = bacc.Bacc(
        "TRN2",
        target_bir_lowering=False,
        debug=False,
        num_devices=plan.NC,
        num_swdge_queues=num_swdge_queues,
        dynamic_dma_scratch_size=49152,
    )
    rg = [list(range(plan.NC))]

    # ---- DRAM I/O ----
    x16 = nc.dram_tensor("x16", [plan.N, P], f16, kind="ExternalInput")
    w16 = nc.dram_tensor("w16", [NL, P, P], f16, kind="ExternalInput")
    b16 = nc.dram_tensor("b16", [1, NL * P], f16, kind="ExternalInput")
    wc32 = nc.dram_tensor("wc32", [P, plan.C], f32, kind="ExternalInput")
    bc32 = nc.dram_tensor("bc32", [1, plan.C], f32, kind="ExternalInput")
    iota16 = nc.dram_tensor("iota16", [P, P], f16, kind="ExternalInput")
    src16 = nc.dram_tensor("src16", [P, plan.slots // 16], i16, kind="ExternalInput")
    dl16 = nc.dram_tensor("dl16", [P, plan.ntiles], f16, kind="ExternalInput")
    scc32 = nc.dram_tensor("scc32", [P, NB], f32, kind="ExternalInput")
    scd32 = nc.dram_tensor("scd32", [P, NB], f32, kind="ExternalInput")
    icd16 = nc.dram_tensor("icd16", [1, NB * P], f16, kind="ExternalInput")
    gid16 = nc.dram_tensor("gid16", [P, NB], f32, kind="ExternalInput")
    ivc16 = nc.dram_tensor("ivc16", [P, NB], f32, kind="ExternalInput")
    out_d = nc.dram_tensor("out", [plan.G, plan.C], f32, kind="ExternalOutput")

    # internal DRAM: per-layer local slice + gathered full table
    hloc = [nc.dram_tensor(f"hloc{l}", [plan.NPC, P], f16) for l in range(NL - 1)]
    hfull = [nc.dram_tensor(f"hfull{l}", [plan.NPC * plan.NC, P], f16,
                            addr_space="Shared") for l in range(NL - 1)]
    pg_in = nc.dram_tensor("pg_in", [P, P], f32)
    pg_out = nc.dram_tensor("pg_out", [P, P], f32, addr_space="Shared")

    with tile.TileContext(nc) as tc, ExitStack() as ctx:
        const = ctx.enter_context(tc.tile_pool(name="const", bufs=1))
        gpool = ctx.enter_context(tc.tile_pool(name="gather", bufs=3))
        spool = ctx.enter_context(tc.tile_pool(name="sel", bufs=6))
        mpool = ctx.enter_context(tc.tile_pool(name="mt", bufs=3))
        hpool = ctx.enter_context(tc.tile_pool(name="hb", bufs=4))
        h4pool = ctx.enter_context(tc.tile_pool(name="h4", bufs=NB))
        psum_m = ctx.enter_context(tc.tile_pool(name="psum_m", bufs=2, space="PSUM"))
        psum_h = ctx.enter_context(tc.tile_pool(name="psum_h", bufs=2, space="PSUM"))
        psum_g = ctx.enter_context(tc.tile_pool(name="psum_g", bufs=1, space="PSUM"))
        opool = ctx.enter_context(tc.tile_pool(name="outp", bufs=1))

        # ---- constants into SBUF ----
        def cload(tag, dram, shape, dt):
            t = const.tile(shape, dt, tag=tag)
            nc.sync.dma_start(out=t[:], in_=dram[:])
            return t

        IOTA = cload("iota", iota16, [P, P], f16)
        SRC = cload("src", src16, [P, plan.slots // 16], i16)
        DL = cload("dl", dl16, [P, plan.ntiles], f16)
        SCC = cload("scc", scc32, [P, NB], f32)
        SCD = cload("scd", scd32, [P, NB], f32)
        ICD = cload("icd", icd16, [1, NB * P], f16)
        GID = cload("gid", gid16, [P, NB], f32)
        IVC = cload("ivc", ivc16, [P, NB], f32)
        WTS = [cload(f"wt{l}", w16[l], [P, P], f16) for l in range(NL)]
        BROW = cload("brow", b16, [1, NL * P], f16)
        WC = cload("wc", wc32, [P, plan.C], f32)
        BC = cload("bc", bc32, [1, plan.C], f32)
        ONE16 = const.tile([1, P], f16, tag="one16")
        nc.vector.memset(ONE16[:], 1.0)
        ONE32 = const.tile([1, P], f32, tag="one32")
        nc.vector.memset(ONE32[:], 1.0)

        is_eq = mybir.AluOpType.is_equal
        mult = mybir.AluOpType.mult

        def sel_tile(col_src, col):
            """Sg^T[n,g] = (iota_g == gid[n]) * invc[n], fp16 (readout only)."""
            st = spool.tile([P, P], f16, tag="sel")
            nc.vector.tensor_scalar(
                out=st[:], in0=IOTA[:],
                scalar1=col_src[0][:, col:col + 1],
                scalar2=col_src[1][:, col:col + 1],
                op0=is_eq, op1=mult)
            return st

        def sel_batch(t0, nt):
            """Binary S^T for nt consecutive tiles: [P, nt, P] fp16."""
            st = spool.tile([P, 9, P], f16, tag="selb")
            nc.vector.tensor_tensor(
                out=st[:, :nt, :],
                in0=IOTA[:].unsqueeze(1).to_broadcast([P, nt, P]),
                in1=DL[:, t0:t0 + nt].unsqueeze(2).to_broadcast([P, nt, P]),
                op=is_eq)
            return st

        prev_cc = None  # collective inst whose output feeds this layer's gathers
        h4_tiles = [None] * NB
        for layer in range(NL):
            table = x16 if layer == 0 else hfull[layer - 1]
            with nc.named_scope(f"conv{layer}"):
                for sb in range(plan.nsb):
                    blocks = list(plan.sb_blocks(sb))
                    nblk = len(blocks)
                    sbt = nblk * KT  # tiles in this superblock
                    g = gpool.tile([P, SB * KT, P], f16, tag="g")
                    t0 = plan.tile_index(blocks[0], 0)
                    # two gathers: half0 tiles then half1 tiles
                    for half, Kh, ntile_h, toff in (
                        (0, K0, nblk * K0, 0),
                        (1, K1, nblk * K1, nblk * K0),
                    ):
                        n_idx = ntile_h * P
                        col0 = (t0 + toff) * P // 16
                        tab = table[0:plan.HALF, :] if half == 0 else \
                            table[plan.HALF:plan.N, :]
                        gi = nc.gpsimd.dma_gather(
                            g[:, toff:toff + ntile_h, :],
                            tab,
                            SRC[:, col0:col0 + n_idx // 16],
                            n_idx, n_idx, P,
                            queue_num=(2 * sb + half) % num_swdge_queues,
                            single_packet=False,
                        )
                        if prev_cc is not None:
                            add_dep_helper(_inst(gi), _inst(prev_cc), reason="gather after AG")

                    for bl, b in enumerate(blocks):
                        pm = psum_m.tile([P, P], f32, tag="pm")
                        st0 = sel_batch(plan.tile_index(b, 0), K0)
                        st1 = sel_batch(plan.tile_index(b, K0), K1)
                        for t in range(KT):
                            st = st0[:, t, :] if t < K0 else st1[:, t - K0, :]
                            gt = (bl * K0 + t) if t < K0 else \
                                (nblk * K0 + bl * K1 + (t - K0))
                            nc.tensor.matmul(
                                out=pm[:], lhsT=g[:, gt, :], rhs=st,
                                start=(t == 0), stop=(t == KT - 1))
                        mt = mpool.tile([P, P], f16, tag="mt")
                        nc.vector.tensor_copy(out=mt[:], in_=pm[:])
                        ph = psum_h.tile([P, P], f32, tag="ph")
                        nc.tensor.matmul(out=ph[:], lhsT=mt[:], rhs=WTS[layer][:],
                                         start=True, stop=False)
                        nc.tensor.matmul(out=ph[:],
                                         lhsT=ICD[0:1, b * P:(b + 1) * P],
                                         rhs=BROW[0:1, layer * P:(layer + 1) * P],
                                         start=False, stop=True)
                        if layer < NL - 1:
                            hb = hpool.tile([P, P], f16, tag="hb")
                        else:
                            hb = h4pool.tile([P, P], f16, tag="h4")
                        scl = SCC if layer < NL - 1 else SCD
                        nc.scalar.activation(
                            out=hb[:], in_=ph[:],
                            func=mybir.ActivationFunctionType.Relu,
                            scale=scl[:, b:b + 1])
                        if layer < NL - 1:
                            rows = min(plan.NPC - b * P, P)
                            nc.sync.dma_start(
                                out=hloc[layer][b * P:b * P + rows, :],
                                in_=hb[:rows, :])
                        else:
                            h4_tiles[b] = hb
            if layer < NL - 1:
                if use_collectives:
                    prev_cc = nc.gpsimd.collective_compute(
                        "AllGather", mybir.AluOpType.bypass,
                        ins=[hloc[layer].ap().opt()], outs=[hfull[layer].ap().opt()],
                        replica_groups=rg)
                else:
                    assert plan.NC == 1
                    prev_cc = nc.sync.dma_start(out=hfull[layer][:, :],
                                                in_=hloc[layer][:, :])

        # ---- readout ----
        with nc.named_scope("readout"):
            pg = psum_g.tile([P, P], f32, tag="pg")
            for b in range(NB):
                sg = sel_tile((GID, IVC), b)
                nc.tensor.matmul(out=pg[:], lhsT=h4_tiles[b][:], rhs=sg[:],
                                 start=(b == 0), stop=(b == NB - 1))
            pgs = opool.tile([P, P], f32, tag="pgs")
            nc.vector.tensor_copy(out=pgs[:], in_=pg[:])
            wr = nc.sync.dma_start(out=pg_in[:, :], in_=pgs[:])
            if use_collectives:
                cc = nc.gpsimd.collective_compute(
                    "AllReduce", mybir.AluOpType.add,
                    ins=[pg_in.ap().opt()], outs=[pg_out.ap().opt()],
                    replica_groups=rg)
            else:
                cc = nc.sync.dma_start(out=pg_out[:, :], in_=pg_in[:, :])
            hgT = opool.tile([P, P], f32, tag="hgT")
            rd = nc.sync.dma_start(out=hgT[:], in_=pg_out[:, :])
            add_dep_helper(_inst(rd), _inst(cc), reason="read after AR")
            po = psum_g.tile([P, plan.C], f32, tag="po")
            nc.tensor.matmul(out=po[:plan.G, :], lhsT=hgT[:, :plan.G], rhs=WC[:],
                             start=True, stop=False)
            nc.tensor.matmul(out=po[:plan.G, :], lhsT=ONE32[0:1, :plan.G], rhs=BC[:],
                             start=False, stop=True)
            ob = opool.tile([P, plan.C], f32, tag="ob")
            nc.vector.tensor_copy(out=ob[:plan.G, :], in_=po[:plan.G, :])
            nc.sync.dma_start(out=out_d[:, :], in_=ob[:plan.G, :])

    nc.compile()
    return nc


# --------------------------------------------------------------------------
# Entry point.
# --------------------------------------------------------------------------

_CACHE = {}


def _get_compiled(plan_key, plan):
    if plan_key not in _CACHE:
        _CACHE[plan_key] = build_nc(plan)
    return _CACHE[plan_key]


def kernel(x, W0, b0, Ws, bs, Wc, bc, edge_index, graph_ids):
    x = np.asarray(x)
    edge_index = np.asarray(edge_index)
    graph_ids = np.asarray(graph_ids)
    W_all = np.concatenate([np.asarray(W0)[None], np.asarray(Ws)], axis=0)
    b_all = np.concatenate([np.asarray(b0)[None], np.asarray(bs)], axis=0)
    Wc, bc = np.asarray(Wc), np.asarray(bc)

    plan = make_plan(x, edge_index, graph_ids)
    key = (plan.N, plan.E, plan.G, plan.K0, plan.K1, plan.n_layers)
    nc = _get_compiled(key, plan)

    from concourse.bass_utils import run_bass_kernel_spmd
    in_maps = build_inputs(plan, x, W_all, b_all, Wc, bc)
    res = run_bass_kernel_spmd(nc, in_maps, core_ids=list(range(plan.NC)))
    return res.results[0]["out"].astype(np.float32)


# revision 16
# speedup vs baseline: 1.4665x; 1.0362x over previous
"""GNN message-passing (GraphConv x4 + mean readout + linear classifier) on 8 TRN2 cores.

Sharding: dst-node (and incident-edge) partitioning across 8 cores. Each layer:
  - every core holds the full node-feature table (fp16) in DRAM (replicated via AllGather)
  - per 128-dst block: dma_gather src rows (fp16), build per-tile selection matrices
    S^T[e,d] = w_e * (dst_local[e]==d) on DVE, aggregate m^T via TensorE into PSUM,
    then h' = relu(m @ W + b) and write the core's slice; AllGather -> next table.
Readout: per-block matmul against graph-selection weights (1/cnt folded in),
AllReduce, then classifier matmul.  Dominant traffic: 256B/edge/layer gather.
"""

import math
from contextlib import ExitStack
from dataclasses import dataclass, field

import numpy as np

P = 128  # partitions; also feature dim and max graph count here


# --------------------------------------------------------------------------
# Planning: pack edges into per-core, per-superblock, per-half tile slots.
#
# Flat slot order per core:
#   for each superblock sb (SB blocks):
#     [all half0 tiles: block b0..b_last, K0 tiles each]
#     [all half1 tiles: block b0..b_last, K1 tiles each]
# Each tile is 128 slots (one gathered edge row per partition).
# --------------------------------------------------------------------------

@dataclass
class Plan:
    N: int
    E: int
    D: int
    C: int
    G: int
    NC: int
    NPC: int
    NB: int
    K0: int
    K1: int
    SB: int
    HALF: int
    n_layers: int
    src16: list = field(default_factory=list)   # [P, slots//16] int16 (row-replicated x8)
    dl: list = field(default_factory=list)      # [P, ntiles] fp16 dst_local (128=pad)
    scc: list = field(default_factory=list)     # [P, NB] fp32 c_src*c_dst per node
    scd: list = field(default_factory=list)     # [P, NB] fp32 c_dst per node
    icd: list = field(default_factory=list)     # [1, NB*P] fp16 1/c_dst per node
    gidf: list = field(default_factory=list)    # [P, NB] fp32 graph id per node
    invc: list = field(default_factory=list)    # [P, NB] fp32 1/cnt per node

    @property
    def nsb(self):
        return math.ceil(self.NB / self.SB)

    def sb_blocks(self, sb):
        return range(sb * self.SB, min((sb + 1) * self.SB, self.NB))

    @property
    def ntiles(self):
        return self.NB * (self.K0 + self.K1)

    @property
    def slots(self):
        return self.ntiles * P

    def tile_index(self, b, t):
        """Global tile index for block b, per-block tile t (t<K0: half0)."""
        sb, bl = b // self.SB, b % self.SB
        nblk = len(self.sb_blocks(sb))
        base = sb * self.SB * (self.K0 + self.K1)  # tiles before this sb
        if t < self.K0:
            return base + bl * self.K0 + t
        return base + nblk * self.K0 + bl * self.K1 + (t - self.K0)


def make_plan(x, edge_index, graph_ids, n_layers=4, NC=8, SB=4, C=10, G=None):
    N, D = x.shape
    E = edge_index.shape[1]
    if G is None:
        G = int(np.asarray(graph_ids).max()) + 1
    assert G <= P and D == P
    src = np.asarray(edge_index[0], dtype=np.int64)
    dst = np.asarray(edge_index[1], dtype=np.int64)
    NPC = math.ceil(N / NC)
    NB = math.ceil(NPC / P)
    HALF = math.ceil(N / 2)
    assert HALF <= 32767 and N - HALF <= 32767

    out_deg = np.bincount(src, minlength=N).astype(np.float64)
    in_deg = np.bincount(dst, minlength=N).astype(np.float64)
    c_src = np.clip(out_deg, 1.0, None) ** -0.5
    c_dst = np.clip(in_deg, 1.0, None) ** -0.5
    w_all = (c_src[src] * c_dst[dst]).astype(np.float32)

    core_of_edge = dst // NPC
    blk_in_core = (dst - core_of_edge * NPC) // P
    half_of_edge = (src >= HALF).astype(np.int64)
    key = np.lexsort((src, half_of_edge, blk_in_core, core_of_edge))
    src_s, w_s = src[key], w_all[key]
    dst_s = dst[key]
    half_s = half_of_edge[key]
    core_s = core_of_edge[key]
    blk_s = blk_in_core[key]

    K0 = K1 = 1
    per_block = {}
    for c in range(NC):
        mc = core_s == c
        for b in range(NB):
            mb = mc & (blk_s == b)
            e0 = int((half_s[mb] == 0).sum())
            e1 = int(mb.sum()) - e0
            per_block[(c, b)] = mb
            K0 = max(K0, math.ceil(max(e0, 1) / P))
            K1 = max(K1, math.ceil(max(e1, 1) / P))

    plan = Plan(N=N, E=E, D=D, C=C, G=G, NC=NC, NPC=NPC, NB=NB, K0=K0, K1=K1,
                SB=SB, HALF=HALF, n_layers=n_layers)

    cnt = np.bincount(np.asarray(graph_ids, dtype=np.int64), minlength=G).astype(np.float64)
    invc_all = 1.0 / np.clip(cnt, 1.0, None)

    # per-node normalization folded into tables/activations:
    #   table_l = c_src * h_l ; h' = relu(cc * (m_raw @ W + invcd x b))
    plan.c_src, plan.c_dst = c_src.astype(np.float32), c_dst.astype(np.float32)
    ntiles = plan.ntiles
    for c in range(NC):
        src_flat = np.zeros(ntiles * P, dtype=np.int16)
        dl_flat = np.full(ntiles * P, 128.0, dtype=np.float16)  # 128 = no match
        for b in range(NB):
            mb = per_block[(c, b)]
            s_b, d_b, h_b = src_s[mb], dst_s[mb], half_s[mb]
            dloc = (d_b - c * NPC - b * P).astype(np.float16)
            for half, K in ((0, K0), (1, K1)):
                sel = h_b == half
                idx = s_b[sel] - (HALF if half else 0)
                t0 = plan.tile_index(b, 0 if half == 0 else K0)
                base = t0 * P
                n = len(idx)
                src_flat[base:base + n] = idx.astype(np.int16)
                dl_flat[base:base + n] = dloc[sel]

        wrapped = src_flat.reshape(-1, 16).T          # [16, slots/16]
        plan.src16.append(np.ascontiguousarray(np.tile(wrapped, (8, 1))))
        plan.dl.append(np.ascontiguousarray(dl_flat.reshape(-1, P).T))   # [P, ntiles]
        # per-block-node scale/bias-fold arrays
        lo, hi = c * NPC, min((c + 1) * NPC, N)
        cs = np.zeros(NB * P, np.float32); cs[:hi - lo] = c_src[lo:hi]
        cd = np.ones(NB * P, np.float32); cd[:hi - lo] = c_dst[lo:hi]
        plan.scc.append(np.ascontiguousarray((cs * cd).reshape(NB, P).T))  # [P,NB]
        plan.scd.append(np.ascontiguousarray(cd.reshape(NB, P).T))         # [P,NB]
        plan.icd.append(np.ascontiguousarray((1.0 / cd).reshape(1, NB * P).astype(np.float16)))

        gidf = np.zeros(NB * P, dtype=np.float32)
        invc = np.zeros(NB * P, dtype=np.float32)
        lo, hi = c * NPC, min((c + 1) * NPC, N)
        gids = np.asarray(graph_ids[lo:hi], dtype=np.int64)
        gidf[:hi - lo] = gids.astype(np.float32)
        invc[:hi - lo] = invc_all[gids].astype(np.float32)
        plan.gidf.append(np.ascontiguousarray(gidf.reshape(NB, P).T))    # [P, NB]
        plan.invc.append(np.ascontiguousarray(invc.reshape(NB, P).T))    # [P, NB]

    return plan


# --------------------------------------------------------------------------
# Golden numpy model of the exact device algorithm (fp16 gather/aggregation).
# --------------------------------------------------------------------------

def golden(plan: Plan, x, W_all, b_all, Wc, bc):
    f16, f32 = np.float16, np.float32
    htab = (x * plan.c_src[:, None]).astype(f16)
    NPC, NB, K0, K1, HALF = plan.NPC, plan.NB, plan.K0, plan.K1, plan.HALF
    KT = K0 + K1
    iota = np.arange(P, dtype=f16)[None, :]
    h4_blocks = [[None] * NB for _ in range(plan.NC)]
    for layer in range(plan.n_layers):
        W16 = W_all[layer].astype(f16)
        nxt = np.zeros((plan.N, plan.D), dtype=f16)
        for c in range(plan.NC):
            flat_idx = plan.src16[c][:16, :].T.reshape(-1)
            dl = plan.dl[c]
            for b in range(NB):
                mT = np.zeros((plan.D, P), dtype=f32)
                for t in range(KT):
                    ti = plan.tile_index(b, t)
                    sl = flat_idx[ti * P:(ti + 1) * P].astype(np.int64)
                    base = 0 if t < K0 else HALF
                    M = htab[base + sl, :]
                    ST = (iota == dl[:, ti:ti + 1]).astype(f16)
                    mT += M.astype(f32).T @ ST.astype(f32)
                mT16 = mT.astype(f16)
                pre = mT16.astype(f32).T @ W16.astype(f32)
                icd = plan.icd[c][0, b * P:(b + 1) * P].astype(f32)
                pre += icd[:, None] @ b_all[layer].astype(f32)[None, :]
                scl = (plan.scc[c] if layer < plan.n_layers - 1 else
                       plan.scd[c])[:, b]
                hb = np.maximum(pre * scl[:, None], 0).astype(f16)
                h4_blocks[c][b] = hb
                lo = c * NPC + b * P
                hi = min(lo + P, min((c + 1) * NPC, plan.N))
                if hi > lo:
                    nxt[lo:hi] = hb[:hi - lo]
        htab = nxt
    pgT = np.zeros((plan.D, P), dtype=f32)
    for c in range(plan.NC):
        for b in range(NB):
            hb = h4_blocks[c][b]
            SgT = ((iota.astype(np.float32) == plan.gidf[c][:, b:b + 1]) * plan.invc[c][:, b:b + 1]).astype(f16)
            pgT += hb.astype(f32).T @ SgT.astype(f32)
    out = pgT.T @ Wc.astype(f32) + bc[None, :]
    return out[:plan.G].astype(f32)


# --------------------------------------------------------------------------
# Bass/Tile kernel builder.
# --------------------------------------------------------------------------

def build_inputs(plan: Plan, x, W_all, b_all, Wc, bc):
    """Per-core in_maps for run_bass_kernel_spmd."""
    iota = np.tile(np.arange(P, dtype=np.float16)[None, :], (P, 1))
    common = {
        "x16": np.ascontiguousarray((x * plan.c_src[:, None]).astype(np.float16)),
        "w16": np.ascontiguousarray(W_all.astype(np.float16)),
        "b16": np.ascontiguousarray(b_all.astype(np.float16).reshape(1, -1)),
        "wc32": np.ascontiguousarray(Wc.astype(np.float32)),
        "bc32": np.ascontiguousarray(bc.astype(np.float32).reshape(1, -1)),
        "iota16": iota,
    }
    maps = []
    for c in range(plan.NC):
        m = dict(common)
        m["src16"] = plan.src16[c]
        m["dl16"] = plan.dl[c]
        m["scc32"] = plan.scc[c]
        m["scd32"] = plan.scd[c]
        m["icd16"] = plan.icd[c]
        m["gid16"] = plan.gidf[c]
        m["ivc16"] = plan.invc[c]
        maps.append(m)
    return maps


def build_nc(plan: Plan, num_swdge_queues=4, use_collectives=True):
    import concourse.bass as bass
    import concourse.tile as tile
    from concourse import bacc, mybir
    from concourse.tile_rust import add_dep_helper

    def _inst(i):
        return i.ins if hasattr(i, "ins") and not hasattr(i, "engine") else i

    f16, f32, i16 = mybir.dt.float16, mybir.dt.float32, mybir.dt.int16
    NB, K0, K1, SB = plan.NB, plan.K0, plan.K1, plan.SB
    KT = K0 + K1
    NL = plan.n_layers

    nc = bacc.Bacc(
        "TRN2",
        target_bir_lowering=False,
        debug=False,
        num_devices=plan.NC,
        num_swdge_queues=num_swdge_queues,
        dynamic_dma_scratch_size=49152,
    )
    rg = [list(range(plan.NC))]

    # ---- DRAM I/O ----
    x16 = nc.dram_tensor("x16", [plan.N, P], f16, kind="ExternalInput")
    w16 = nc.dram_tensor("w16", [NL, P, P], f16, kind="ExternalInput")
    b16 = nc.dram_tensor("b16", [1, NL * P], f16, kind="ExternalInput")
    wc32 = nc.dram_tensor("wc32", [P, plan.C], f32, kind="ExternalInput")
    bc32 = nc.dram_tensor("bc32", [1, plan.C], f32, kind="ExternalInput")
    iota16 = nc.dram_tensor("iota16", [P, P], f16, kind="ExternalInput")
    src16 = nc.dram_tensor("src16", [P, plan.slots // 16], i16, kind="ExternalInput")
    dl16 = nc.dram_tensor("dl16", [P, plan.ntiles], f16, kind="ExternalInput")
    scc32 = nc.dram_tensor("scc32", [P, NB], f32, kind="ExternalInput")
    scd32 = nc.dram_tensor("scd32", [P, NB], f32, kind="ExternalInput")
    icd16 = nc.dram_tensor("icd16", [1, NB * P], f16, kind="ExternalInput")
    gid16 = nc.dram_tensor("gid16", [P, NB], f32, kind="ExternalInput")
    ivc16 = nc.dram_tensor("ivc16", [P, NB], f32, kind="ExternalInput")
    out_d = nc.dram_tensor("out", [plan.G, plan.C], f32, kind="ExternalOutput")

    # internal DRAM: per-layer local slice + gathered full table
    hloc = [nc.dram_tensor(f"hloc{l}", [plan.NPC, P], f16) for l in range(NL - 1)]
    hfull = [nc.dram_tensor(f"hfull{l}", [plan.NPC * plan.NC, P], f16,
                            addr_space="Shared") for l in range(NL - 1)]
    pg_in = nc.dram_tensor("pg_in", [P, P], f32)
    pg_out = nc.dram_tensor("pg_out", [P, P], f32, addr_space="Shared")

    with tile.TileContext(nc) as tc, ExitStack() as ctx:
        const = ctx.enter_context(tc.tile_pool(name="const", bufs=1))
        gpool = ctx.enter_context(tc.tile_pool(name="gather", bufs=5))
        spool = ctx.enter_context(tc.tile_pool(name="sel", bufs=8))
        mpool = ctx.enter_context(tc.tile_pool(name="mt", bufs=4))
        hpool = ctx.enter_context(tc.tile_pool(name="hb", bufs=4))
        h4pool = ctx.enter_context(tc.tile_pool(name="h4", bufs=NB))
        psum_m = ctx.enter_context(tc.tile_pool(name="psum_m", bufs=3, space="PSUM"))
        psum_h = ctx.enter_context(tc.tile_pool(name="psum_h", bufs=2, space="PSUM"))
        psum_g = ctx.enter_context(tc.tile_pool(name="psum_g", bufs=1, space="PSUM"))
        opool = ctx.enter_context(tc.tile_pool(name="outp", bufs=1))

        # ---- constants into SBUF ----
        def cload(tag, dram, shape, dt):
            t = const.tile(shape, dt, tag=tag)
            nc.sync.dma_start(out=t[:], in_=dram[:])
            return t

        IOTA = cload("iota", iota16, [P, P], f16)
        SRC = cload("src", src16, [P, plan.slots // 16], i16)
        DL = cload("dl", dl16, [P, plan.ntiles], f16)
        SCC = cload("scc", scc32, [P, NB], f32)
        SCD = cload("scd", scd32, [P, NB], f32)
        ICD = cload("icd", icd16, [1, NB * P], f16)
        GID = cload("gid", gid16, [P, NB], f32)
        IVC = cload("ivc", ivc16, [P, NB], f32)
        WTS = [cload(f"wt{l}", w16[l], [P, P], f16) for l in range(NL)]
        BROW = cload("brow", b16, [1, NL * P], f16)
        WC = cload("wc", wc32, [P, plan.C], f32)
        BC = cload("bc", bc32, [1, plan.C], f32)
        ONE16 = const.tile([1, P], f16, tag="one16")
        nc.vector.memset(ONE16[:], 1.0)
        ONE32 = const.tile([1, P], f32, tag="one32")
        nc.vector.memset(ONE32[:], 1.0)

        is_eq = mybir.AluOpType.is_equal
        mult = mybir.AluOpType.mult

        def sel_tile(col_src, col):
            """Sg^T[n,g] = (iota_g == gid[n]) * invc[n], fp16 (readout only)."""
            st = spool.tile([P, P], f16, tag="sel")
            nc.vector.tensor_scalar(
                out=st[:], in0=IOTA[:],
                scalar1=col_src[0][:, col:col + 1],
                scalar2=col_src[1][:, col:col + 1],
                op0=is_eq, op1=mult)
            return st

        def sel_batch(t0, nt):
            """Binary S^T for nt consecutive tiles: [P, nt, P] fp16."""
            st = spool.tile([P, 9, P], f16, tag="selb")
            nc.vector.tensor_tensor(
                out=st[:, :nt, :],
                in0=IOTA[:].unsqueeze(1).to_broadcast([P, nt, P]),
                in1=DL[:, t0:t0 + nt].unsqueeze(2).to_broadcast([P, nt, P]),
                op=is_eq)
            return st

        prev_cc = None  # collective inst whose output feeds this layer's gathers
        h4_tiles = [None] * NB
        for layer in range(NL):
            table = x16 if layer == 0 else hfull[layer - 1]
            with nc.named_scope(f"conv{layer}"):
                for sb in range(plan.nsb):
                    blocks = list(plan.sb_blocks(sb))
                    nblk = len(blocks)
                    sbt = nblk * KT  # tiles in this superblock
                    g = gpool.tile([P, SB * KT, P], f16, tag="g")
                    t0 = plan.tile_index(blocks[0], 0)
                    # two gathers: half0 tiles then half1 tiles
                    for half, Kh, ntile_h, toff in (
                        (0, K0, nblk * K0, 0),
                        (1, K1, nblk * K1, nblk * K0),
                    ):
                        n_idx = ntile_h * P
                        col0 = (t0 + toff) * P // 16
                        tab = table[0:plan.HALF, :] if half == 0 else \
                            table[plan.HALF:plan.N, :]
                        gi = nc.gpsimd.dma_gather(
                            g[:, toff:toff + ntile_h, :],
                            tab,
                            SRC[:, col0:col0 + n_idx // 16],
                            n_idx, n_idx, P,
                            queue_num=(2 * sb + half) % num_swdge_queues,
                            single_packet=False,
                        )
                        if prev_cc is not None:
                            add_dep_helper(_inst(gi), _inst(prev_cc), reason="gather after AG")

                    for bl, b in enumerate(blocks):
                        pm = psum_m.tile([P, P], f32, tag="pm")
                        st0 = sel_batch(plan.tile_index(b, 0), K0)
                        st1 = sel_batch(plan.tile_index(b, K0), K1)
                        for t in range(KT):
                            st = st0[:, t, :] if t < K0 else st1[:, t - K0, :]
                            gt = (bl * K0 + t) if t < K0 else \
                                (nblk * K0 + bl * K1 + (t - K0))
                            nc.tensor.matmul(
                                out=pm[:], lhsT=g[:, gt, :], rhs=st,
                                start=(t == 0), stop=(t == KT - 1))
                        mt = mpool.tile([P, P], f16, tag="mt")
                        nc.vector.tensor_copy(out=mt[:], in_=pm[:])
                        ph = psum_h.tile([P, P], f32, tag="ph")
                        nc.tensor.matmul(out=ph[:], lhsT=mt[:], rhs=WTS[layer][:],
                                         start=True, stop=False)
                        nc.tensor.matmul(out=ph[:],
                                         lhsT=ICD[0:1, b * P:(b + 1) * P],
                                         rhs=BROW[0:1, layer * P:(layer + 1) * P],
                                         start=False, stop=True)
                        if layer < NL - 1:
                            hb = hpool.tile([P, P], f16, tag="hb")
                        else:
                            hb = h4pool.tile([P, P], f16, tag="h4")
                        scl = SCC if layer < NL - 1 else SCD
                        nc.scalar.activation(
                            out=hb[:], in_=ph[:],
                            func=mybir.ActivationFunctionType.Relu,
                            scale=scl[:, b:b + 1])
                        if layer < NL - 1:
                            rows = min(plan.NPC - b * P, P)
                            nc.sync.dma_start(
                                out=hloc[layer][b * P:b * P + rows, :],
                                in_=hb[:rows, :])
                        else:
                            h4_tiles[b] = hb
            if layer < NL - 1:
                if use_collectives:
                    prev_cc = nc.gpsimd.collective_compute(
                        "AllGather", mybir.AluOpType.bypass,
                        ins=[hloc[layer].ap().opt()], outs=[hfull[layer].ap().opt()],
                        replica_groups=rg)
                else:
                    assert plan.NC == 1
                    prev_cc = nc.sync.dma_start(out=hfull[layer][:, :],
                                                in_=hloc[layer][:, :])

        # ---- readout ----
        with nc.named_scope("readout"):
            pg = psum_g.tile([P, P], f32, tag="pg")
            for b in range(NB):
                sg = sel_tile((GID, IVC), b)
                nc.tensor.matmul(out=pg[:], lhsT=h4_tiles[b][:], rhs=sg[:],
                                 start=(b == 0), stop=(b == NB - 1))
            pgs = opool.tile([P, P], f32, tag="pgs")
            nc.vector.tensor_copy(out=pgs[:], in_=pg[:])
            wr = nc.sync.dma_start(out=pg_in[:, :], in_=pgs[:])
            if use_collectives:
                cc = nc.gpsimd.collective_compute(
                    "AllReduce", mybir.AluOpType.add,
                    ins=[pg_in.ap().opt()], outs=[pg_out.ap().opt()],
                    replica_groups=rg)
            else:
                cc = nc.sync.dma_start(out=pg_out[:, :], in_=pg_in[:, :])
            hgT = opool.tile([P, P], f32, tag="hgT")
            rd = nc.sync.dma_start(out=hgT[:], in_=pg_out[:, :])
            add_dep_helper(_inst(rd), _inst(cc), reason="read after AR")
            po = psum_g.tile([P, plan.C], f32, tag="po")
            nc.tensor.matmul(out=po[:plan.G, :], lhsT=hgT[:, :plan.G], rhs=WC[:],
                             start=True, stop=False)
            nc.tensor.matmul(out=po[:plan.G, :], lhsT=ONE32[0:1, :plan.G], rhs=BC[:],
                             start=False, stop=True)
            ob = opool.tile([P, plan.C], f32, tag="ob")
            nc.vector.tensor_copy(out=ob[:plan.G, :], in_=po[:plan.G, :])
            nc.sync.dma_start(out=out_d[:, :], in_=ob[:plan.G, :])

    nc.compile()
    return nc


# --------------------------------------------------------------------------
# Entry point.
# --------------------------------------------------------------------------

_CACHE = {}


def _get_compiled(plan_key, plan):
    if plan_key not in _CACHE:
        _CACHE[plan_key] = build_nc(plan)
    return _CACHE[plan_key]


def kernel(x, W0, b0, Ws, bs, Wc, bc, edge_index, graph_ids):
    x = np.asarray(x)
    edge_index = np.asarray(edge_index)
    graph_ids = np.asarray(graph_ids)
    W_all = np.concatenate([np.asarray(W0)[None], np.asarray(Ws)], axis=0)
    b_all = np.concatenate([np.asarray(b0)[None], np.asarray(bs)], axis=0)
    Wc, bc = np.asarray(Wc), np.asarray(bc)

    plan = make_plan(x, edge_index, graph_ids)
    key = (plan.N, plan.E, plan.G, plan.K0, plan.K1, plan.n_layers)
    nc = _get_compiled(key, plan)

    from concourse.bass_utils import run_bass_kernel_spmd
    in_maps = build_inputs(plan, x, W_all, b_all, Wc, bc)
    res = run_bass_kernel_spmd(nc, in_maps, core_ids=list(range(plan.NC)))
    return res.results[0]["out"].astype(np.float32)


# revision 18
# speedup vs baseline: 1.4903x; 1.0162x over previous
"""GNN message-passing (GraphConv x4 + mean readout + linear classifier) on 8 TRN2 cores.

Sharding: dst-node (and incident-edge) partitioning across 8 cores. Each layer:
  - every core holds the full node-feature table (fp16) in DRAM (replicated via AllGather)
  - per 128-dst block: dma_gather src rows (fp16), build per-tile selection matrices
    S^T[e,d] = w_e * (dst_local[e]==d) on DVE, aggregate m^T via TensorE into PSUM,
    then h' = relu(m @ W + b) and write the core's slice; AllGather -> next table.
Readout: per-block matmul against graph-selection weights (1/cnt folded in),
AllReduce, then classifier matmul.  Dominant traffic: 256B/edge/layer gather.
"""

import math
from contextlib import ExitStack
from dataclasses import dataclass, field

import numpy as np

P = 128  # partitions; also feature dim and max graph count here


# --------------------------------------------------------------------------
# Planning: pack edges into per-core, per-superblock, per-half tile slots.
#
# Flat slot order per core:
#   for each superblock sb (SB blocks):
#     [all half0 tiles: block b0..b_last, K0 tiles each]
#     [all half1 tiles: block b0..b_last, K1 tiles each]
# Each tile is 128 slots (one gathered edge row per partition).
# --------------------------------------------------------------------------

@dataclass
class Plan:
    N: int
    E: int
    D: int
    C: int
    G: int
    NC: int
    NPC: int
    NB: int
    K0: int
    K1: int
    SB: int
    HALF: int
    n_layers: int
    src16: list = field(default_factory=list)   # [P, slots//16] int16 (row-replicated x8)
    dl: list = field(default_factory=list)      # [P, ntiles] fp16 dst_local (128=pad)
    scc: list = field(default_factory=list)     # [P, NB] fp32 c_src*c_dst per node
    scd: list = field(default_factory=list)     # [P, NB] fp32 c_dst per node
    icd: list = field(default_factory=list)     # [1, NB*P] fp16 1/c_dst per node
    gidf: list = field(default_factory=list)    # [P, NB] fp32 graph id per node
    invc: list = field(default_factory=list)    # [P, NB] fp32 1/cnt per node

    @property
    def nsb(self):
        return math.ceil(self.NB / self.SB)

    def sb_blocks(self, sb):
        return range(sb * self.SB, min((sb + 1) * self.SB, self.NB))

    @property
    def ntiles(self):
        return self.NB * (self.K0 + self.K1)

    @property
    def slots(self):
        return self.ntiles * P

    def tile_index(self, b, t):
        """Global tile index for block b, per-block tile t (t<K0: half0)."""
        sb, bl = b // self.SB, b % self.SB
        nblk = len(self.sb_blocks(sb))
        base = sb * self.SB * (self.K0 + self.K1)  # tiles before this sb
        if t < self.K0:
            return base + bl * self.K0 + t
        return base + nblk * self.K0 + bl * self.K1 + (t - self.K0)


def make_plan(x, edge_index, graph_ids, n_layers=4, NC=8, SB=4, C=10, G=None):
    N, D = x.shape
    E = edge_index.shape[1]
    if G is None:
        G = int(np.asarray(graph_ids).max()) + 1
    assert G <= P and D == P
    src = np.asarray(edge_index[0], dtype=np.int64)
    dst = np.asarray(edge_index[1], dtype=np.int64)
    NPC = math.ceil(N / NC)
    NB = math.ceil(NPC / P)
    HALF = math.ceil(N / 2)
    assert HALF <= 32767 and N - HALF <= 32767

    out_deg = np.bincount(src, minlength=N).astype(np.float64)
    in_deg = np.bincount(dst, minlength=N).astype(np.float64)
    c_src = np.clip(out_deg, 1.0, None) ** -0.5
    c_dst = np.clip(in_deg, 1.0, None) ** -0.5
    w_all = (c_src[src] * c_dst[dst]).astype(np.float32)

    core_of_edge = dst // NPC
    blk_in_core = (dst - core_of_edge * NPC) // P
    half_of_edge = (src >= HALF).astype(np.int64)
    key = np.lexsort((src, half_of_edge, blk_in_core, core_of_edge))
    src_s, w_s = src[key], w_all[key]
    dst_s = dst[key]
    half_s = half_of_edge[key]
    core_s = core_of_edge[key]
    blk_s = blk_in_core[key]

    K0 = K1 = 1
    per_block = {}
    for c in range(NC):
        mc = core_s == c
        for b in range(NB):
            mb = mc & (blk_s == b)
            e0 = int((half_s[mb] == 0).sum())
            e1 = int(mb.sum()) - e0
            per_block[(c, b)] = mb
            K0 = max(K0, math.ceil(max(e0, 1) / P))
            K1 = max(K1, math.ceil(max(e1, 1) / P))

    plan = Plan(N=N, E=E, D=D, C=C, G=G, NC=NC, NPC=NPC, NB=NB, K0=K0, K1=K1,
                SB=SB, HALF=HALF, n_layers=n_layers)

    cnt = np.bincount(np.asarray(graph_ids, dtype=np.int64), minlength=G).astype(np.float64)
    invc_all = 1.0 / np.clip(cnt, 1.0, None)

    # per-node normalization folded into tables/activations:
    #   table_l = c_src * h_l ; h' = relu(cc * (m_raw @ W + invcd x b))
    plan.c_src, plan.c_dst = c_src.astype(np.float32), c_dst.astype(np.float32)
    ntiles = plan.ntiles
    for c in range(NC):
        src_flat = np.zeros(ntiles * P, dtype=np.int16)
        dl_flat = np.full(ntiles * P, 128.0, dtype=np.float16)  # 128 = no match
        for b in range(NB):
            mb = per_block[(c, b)]
            s_b, d_b, h_b = src_s[mb], dst_s[mb], half_s[mb]
            dloc = (d_b - c * NPC - b * P).astype(np.float16)
            for half, K in ((0, K0), (1, K1)):
                sel = h_b == half
                idx = s_b[sel] - (HALF if half else 0)
                t0 = plan.tile_index(b, 0 if half == 0 else K0)
                base = t0 * P
                n = len(idx)
                src_flat[base:base + n] = idx.astype(np.int16)
                dl_flat[base:base + n] = dloc[sel]

        wrapped = src_flat.reshape(-1, 16).T          # [16, slots/16]
        plan.src16.append(np.ascontiguousarray(np.tile(wrapped, (8, 1))))
        plan.dl.append(np.ascontiguousarray(dl_flat.reshape(-1, P).T))   # [P, ntiles]
        # per-block-node scale/bias-fold arrays
        lo, hi = c * NPC, min((c + 1) * NPC, N)
        cs = np.zeros(NB * P, np.float32); cs[:hi - lo] = c_src[lo:hi]
        cd = np.ones(NB * P, np.float32); cd[:hi - lo] = c_dst[lo:hi]
        plan.scc.append(np.ascontiguousarray((cs * cd).reshape(NB, P).T))  # [P,NB]
        plan.scd.append(np.ascontiguousarray(cd.reshape(NB, P).T))         # [P,NB]
        plan.icd.append(np.ascontiguousarray((1.0 / cd).reshape(1, NB * P).astype(np.float16)))

        gidf = np.zeros(NB * P, dtype=np.float32)
        invc = np.zeros(NB * P, dtype=np.float32)
        lo, hi = c * NPC, min((c + 1) * NPC, N)
        gids = np.asarray(graph_ids[lo:hi], dtype=np.int64)
        gidf[:hi - lo] = gids.astype(np.float32)
        invc[:hi - lo] = invc_all[gids].astype(np.float32)
        plan.gidf.append(np.ascontiguousarray(gidf.reshape(NB, P).T))    # [P, NB]
        plan.invc.append(np.ascontiguousarray(invc.reshape(NB, P).T))    # [P, NB]

    return plan


# --------------------------------------------------------------------------
# Golden numpy model of the exact device algorithm (fp16 gather/aggregation).
# --------------------------------------------------------------------------

def golden(plan: Plan, x, W_all, b_all, Wc, bc):
    f16, f32 = np.float16, np.float32
    htab = (x * plan.c_src[:, None]).astype(f16)
    NPC, NB, K0, K1, HALF = plan.NPC, plan.NB, plan.K0, plan.K1, plan.HALF
    KT = K0 + K1
    iota = np.arange(P, dtype=f16)[None, :]
    h4_blocks = [[None] * NB for _ in range(plan.NC)]
    for layer in range(plan.n_layers):
        W16 = W_all[layer].astype(f16)
        nxt = np.zeros((plan.N, plan.D), dtype=f16)
        for c in range(plan.NC):
            flat_idx = plan.src16[c][:16, :].T.reshape(-1)
            dl = plan.dl[c]
            for b in range(NB):
                mT = np.zeros((plan.D, P), dtype=f32)
                for t in range(KT):
                    ti = plan.tile_index(b, t)
                    sl = flat_idx[ti * P:(ti + 1) * P].astype(np.int64)
                    base = 0 if t < K0 else HALF
                    M = htab[base + sl, :]
                    ST = (iota == dl[:, ti:ti + 1]).astype(f16)
                    mT += M.astype(f32).T @ ST.astype(f32)
                mT16 = mT.astype(f16)
                pre = mT16.astype(f32).T @ W16.astype(f32)
                icd = plan.icd[c][0, b * P:(b + 1) * P].astype(f32)
                pre += icd[:, None] @ b_all[layer].astype(f32)[None, :]
                scl = (plan.scc[c] if layer < plan.n_layers - 1 else
                       plan.scd[c])[:, b]
                hb = np.maximum(pre * scl[:, None], 0).astype(f16)
                h4_blocks[c][b] = hb
                lo = c * NPC + b * P
                hi = min(lo + P, min((c + 1) * NPC, plan.N))
                if hi > lo:
                    nxt[lo:hi] = hb[:hi - lo]
        htab = nxt
    pgT = np.zeros((plan.D, P), dtype=f32)
    for c in range(plan.NC):
        for b in range(NB):
            hb = h4_blocks[c][b]
            SgT = ((iota.astype(np.float32) == plan.gidf[c][:, b:b + 1]) * plan.invc[c][:, b:b + 1]).astype(f16)
            pgT += hb.astype(f32).T @ SgT.astype(f32)
    out = pgT.T @ Wc.astype(f32) + bc[None, :]
    return out[:plan.G].astype(f32)


# --------------------------------------------------------------------------
# Bass/Tile kernel builder.
# --------------------------------------------------------------------------

def build_inputs(plan: Plan, x, W_all, b_all, Wc, bc):
    """Per-core in_maps for run_bass_kernel_spmd."""
    iota = np.tile(np.arange(P, dtype=np.float16)[None, :], (P, 1))
    common = {
        "x16": np.ascontiguousarray((x * plan.c_src[:, None]).astype(np.float16)),
        "w16": np.ascontiguousarray(W_all.astype(np.float16)),
        "b16": np.ascontiguousarray(b_all.astype(np.float16).reshape(1, -1)),
        "wc32": np.ascontiguousarray(Wc.astype(np.float32)),
        "bc32": np.ascontiguousarray(bc.astype(np.float32).reshape(1, -1)),
        "iota16": iota,
    }
    maps = []
    for c in range(plan.NC):
        m = dict(common)
        m["src16"] = plan.src16[c]
        m["dl16"] = plan.dl[c]
        m["scc32"] = plan.scc[c]
        m["scd32"] = plan.scd[c]
        m["icd16"] = plan.icd[c]
        m["gid16"] = plan.gidf[c]
        m["ivc16"] = plan.invc[c]
        maps.append(m)
    return maps


def build_nc(plan: Plan, num_swdge_queues=4, use_collectives=True):
    import concourse.bass as bass
    import concourse.tile as tile
    from concourse import bacc, mybir
    from concourse.tile_rust import add_dep_helper

    def _inst(i):
        return i.ins if hasattr(i, "ins") and not hasattr(i, "engine") else i

    f16, f32, i16 = mybir.dt.float16, mybir.dt.float32, mybir.dt.int16
    NB, K0, K1, SB = plan.NB, plan.K0, plan.K1, plan.SB
    KT = K0 + K1
    NL = plan.n_layers

    nc = bacc.Bacc(
        "TRN2",
        target_bir_lowering=False,
        debug=False,
        num_devices=plan.NC,
        num_swdge_queues=num_swdge_queues,
        dynamic_dma_scratch_size=49152,
    )
    rg = [list(range(plan.NC))]

    # ---- DRAM I/O ----
    x16 = nc.dram_tensor("x16", [plan.N, P], f16, kind="ExternalInput")
    w16 = nc.dram_tensor("w16", [NL, P, P], f16, kind="ExternalInput")
    b16 = nc.dram_tensor("b16", [1, NL * P], f16, kind="ExternalInput")
    wc32 = nc.dram_tensor("wc32", [P, plan.C], f32, kind="ExternalInput")
    bc32 = nc.dram_tensor("bc32", [1, plan.C], f32, kind="ExternalInput")
    iota16 = nc.dram_tensor("iota16", [P, P], f16, kind="ExternalInput")
    src16 = nc.dram_tensor("src16", [P, plan.slots // 16], i16, kind="ExternalInput")
    dl16 = nc.dram_tensor("dl16", [P, plan.ntiles], f16, kind="ExternalInput")
    scc32 = nc.dram_tensor("scc32", [P, NB], f32, kind="ExternalInput")
    scd32 = nc.dram_tensor("scd32", [P, NB], f32, kind="ExternalInput")
    icd16 = nc.dram_tensor("icd16", [1, NB * P], f16, kind="ExternalInput")
    gid16 = nc.dram_tensor("gid16", [P, NB], f32, kind="ExternalInput")
    ivc16 = nc.dram_tensor("ivc16", [P, NB], f32, kind="ExternalInput")
    out_d = nc.dram_tensor("out", [plan.G, plan.C], f32, kind="ExternalOutput")

    # internal DRAM: per-layer local slice + gathered full table
    hloc = [nc.dram_tensor(f"hloc{l}", [plan.NPC, P], f16) for l in range(NL - 1)]
    hfull = [nc.dram_tensor(f"hfull{l}", [plan.NPC * plan.NC, P], f16,
                            addr_space="Shared") for l in range(NL - 1)]
    pg_in = nc.dram_tensor("pg_in", [P, P], f32)
    pg_out = nc.dram_tensor("pg_out", [P, P], f32, addr_space="Shared")

    with tile.TileContext(nc) as tc, ExitStack() as ctx:
        const = ctx.enter_context(tc.tile_pool(name="const", bufs=1))
        gpool = ctx.enter_context(tc.tile_pool(name="gather", bufs=5))
        spool = ctx.enter_context(tc.tile_pool(name="sel", bufs=8))
        mpool = ctx.enter_context(tc.tile_pool(name="mt", bufs=4))
        hpool = ctx.enter_context(tc.tile_pool(name="hb", bufs=4))
        h4pool = ctx.enter_context(tc.tile_pool(name="h4", bufs=NB))
        psum_m = ctx.enter_context(tc.tile_pool(name="psum_m", bufs=3, space="PSUM"))
        psum_h = ctx.enter_context(tc.tile_pool(name="psum_h", bufs=2, space="PSUM"))
        psum_g = ctx.enter_context(tc.tile_pool(name="psum_g", bufs=1, space="PSUM"))
        opool = ctx.enter_context(tc.tile_pool(name="outp", bufs=1))

        # ---- constants into SBUF ----
        def cload(tag, dram, shape, dt):
            t = const.tile(shape, dt, tag=tag)
            nc.sync.dma_start(out=t[:], in_=dram[:])
            return t

        IOTA = cload("iota", iota16, [P, P], f16)
        SRC = cload("src", src16, [P, plan.slots // 16], i16)
        DL = cload("dl", dl16, [P, plan.ntiles], f16)
        SCC = cload("scc", scc32, [P, NB], f32)
        SCD = cload("scd", scd32, [P, NB], f32)
        ICD = cload("icd", icd16, [1, NB * P], f16)
        GID = cload("gid", gid16, [P, NB], f32)
        IVC = cload("ivc", ivc16, [P, NB], f32)
        WTS = [cload(f"wt{l}", w16[l], [P, P], f16) for l in range(NL)]
        BROW = cload("brow", b16, [1, NL * P], f16)
        WC = cload("wc", wc32, [P, plan.C], f32)
        BC = cload("bc", bc32, [1, plan.C], f32)
        ONE16 = const.tile([1, P], f16, tag="one16")
        nc.vector.memset(ONE16[:], 1.0)
        ONE32 = const.tile([1, P], f32, tag="one32")
        nc.vector.memset(ONE32[:], 1.0)

        is_eq = mybir.AluOpType.is_equal
        mult = mybir.AluOpType.mult

        def sel_tile(col_src, col):
            """Sg^T[n,g] = (iota_g == gid[n]) * invc[n], fp16 (readout only)."""
            st = spool.tile([P, P], f16, tag="sel")
            nc.vector.tensor_scalar(
                out=st[:], in0=IOTA[:],
                scalar1=col_src[0][:, col:col + 1],
                scalar2=col_src[1][:, col:col + 1],
                op0=is_eq, op1=mult)
            return st

        def sel_batch(t0, nt):
            """Binary S^T for nt consecutive tiles: [P, nt, P] fp16."""
            st = spool.tile([P, 9, P], f16, tag="selb")
            nc.vector.tensor_tensor(
                out=st[:, :nt, :],
                in0=IOTA[:].unsqueeze(1).to_broadcast([P, nt, P]),
                in1=DL[:, t0:t0 + nt].unsqueeze(2).to_broadcast([P, nt, P]),
                op=is_eq)
            return st

        prev_cc = None  # collective inst whose output feeds this layer's gathers
        h4_tiles = [None] * NB
        for layer in range(NL):
            table = x16 if layer == 0 else hfull[layer - 1]
            with nc.named_scope(f"conv{layer}"):
                for sb in range(plan.nsb):
                    blocks = list(plan.sb_blocks(sb))
                    nblk = len(blocks)
                    sbt = nblk * KT  # tiles in this superblock
                    g = gpool.tile([P, SB * KT, P], f16, tag="g")
                    t0 = plan.tile_index(blocks[0], 0)
                    # two gathers: half0 tiles then half1 tiles
                    for half, Kh, ntile_h, toff in (
                        (0, K0, nblk * K0, 0),
                        (1, K1, nblk * K1, nblk * K0),
                    ):
                        n_idx = ntile_h * P
                        col0 = (t0 + toff) * P // 16
                        tab = table[0:plan.HALF, :] if half == 0 else \
                            table[plan.HALF:plan.N, :]
                        gi = nc.gpsimd.dma_gather(
                            g[:, toff:toff + ntile_h, :],
                            tab,
                            SRC[:, col0:col0 + n_idx // 16],
                            n_idx, n_idx, P,
                            queue_num=(2 * sb + half) % num_swdge_queues,
                            single_packet=False,
                        )
                        if prev_cc is not None:
                            add_dep_helper(_inst(gi), _inst(prev_cc), reason="gather after AG")

                    for bl, b in enumerate(blocks):
                        pm = psum_m.tile([P, P], f32, tag="pm")
                        st0 = sel_batch(plan.tile_index(b, 0), K0)
                        st1 = sel_batch(plan.tile_index(b, K0), K1)
                        for t in range(KT):
                            st = st0[:, t, :] if t < K0 else st1[:, t - K0, :]
                            gt = (bl * K0 + t) if t < K0 else \
                                (nblk * K0 + bl * K1 + (t - K0))
                            nc.tensor.matmul(
                                out=pm[:], lhsT=g[:, gt, :], rhs=st,
                                start=(t == 0), stop=(t == KT - 1))
                        mt = mpool.tile([P, P], f16, tag="mt")
                        nc.vector.tensor_copy(out=mt[:], in_=pm[:])
                        ph = psum_h.tile([P, P], f32, tag="ph")
                        nc.tensor.matmul(out=ph[:], lhsT=mt[:], rhs=WTS[layer][:],
                                         start=True, stop=False)
                        nc.tensor.matmul(out=ph[:],
                                         lhsT=ICD[0:1, b * P:(b + 1) * P],
                                         rhs=BROW[0:1, layer * P:(layer + 1) * P],
                                         start=False, stop=True)
                        if layer < NL - 1:
                            hb = hpool.tile([P, P], f16, tag="hb")
                        else:
                            hb = h4pool.tile([P, P], f16, tag="h4")
                        scl = SCC if layer < NL - 1 else SCD
                        nc.scalar.activation(
                            out=hb[:], in_=ph[:],
                            func=mybir.ActivationFunctionType.Relu,
                            scale=scl[:, b:b + 1])
                        if layer < NL - 1:
                            rows = min(plan.NPC - b * P, P)
                            nc.sync.dma_start(
                                out=hloc[layer][b * P:b * P + rows, :],
                                in_=hb[:rows, :])
                        else:
                            h4_tiles[b] = hb
            if layer < NL - 1:
                if use_collectives:
                    prev_cc = nc.gpsimd.collective_compute(
                        "AllGather", mybir.AluOpType.bypass,
                        ins=[hloc[layer].ap().opt()], outs=[hfull[layer].ap().opt()],
                        replica_groups=rg)
                else:
                    assert plan.NC == 1
                    prev_cc = nc.sync.dma_start(out=hfull[layer][:, :],
                                                in_=hloc[layer][:, :])

        # ---- readout ----
        with nc.named_scope("readout"):
            pg = psum_g.tile([P, P], f32, tag="pg")
            for b in range(NB):
                sg = sel_tile((GID, IVC), b)
                nc.tensor.matmul(out=pg[:], lhsT=h4_tiles[b][:], rhs=sg[:],
                                 start=(b == 0), stop=(b == NB - 1))
            pgs = opool.tile([P, P], f32, tag="pgs")
            nc.vector.tensor_copy(out=pgs[:], in_=pg[:])
            wr = nc.sync.dma_start(out=pg_in[:, :], in_=pgs[:])
            if use_collectives:
                cc = nc.gpsimd.collective_compute(
                    "AllReduce", mybir.AluOpType.add,
                    ins=[pg_in.ap().opt()], outs=[pg_out.ap().opt()],
                    replica_groups=rg)
            else:
                cc = nc.sync.dma_start(out=pg_out[:, :], in_=pg_in[:, :])
            hgT = opool.tile([P, P], f32, tag="hgT")
            rd = nc.sync.dma_start(out=hgT[:], in_=pg_out[:, :])
            add_dep_helper(_inst(rd), _inst(cc), reason="read after AR")
            po = psum_g.tile([P, plan.C], f32, tag="po")
            nc.tensor.matmul(out=po[:plan.G, :], lhsT=hgT[:, :plan.G], rhs=WC[:],
                             start=True, stop=False)
            nc.tensor.matmul(out=po[:plan.G, :], lhsT=ONE32[0:1, :plan.G], rhs=BC[:],
                             start=False, stop=True)
            ob = opool.tile([P, plan.C], f32, tag="ob")
            nc.vector.tensor_copy(out=ob[:plan.G, :], in_=po[:plan.G, :])
            nc.sync.dma_start(out=out_d[:, :], in_=ob[:plan.G, :])

    nc.compile()
    return nc


# --------------------------------------------------------------------------
# Entry point.
# --------------------------------------------------------------------------

_CACHE = {}


def _get_compiled(plan_key, plan):
    if plan_key not in _CACHE:
        _CACHE[plan_key] = build_nc(plan)
    return _CACHE[plan_key]


def kernel(x, W0, b0, Ws, bs, Wc, bc, edge_index, graph_ids):
    x = np.asarray(x)
    edge_index = np.asarray(edge_index)
    graph_ids = np.asarray(graph_ids)
    W_all = np.concatenate([np.asarray(W0)[None], np.asarray(Ws)], axis=0)
    b_all = np.concatenate([np.asarray(b0)[None], np.asarray(bs)], axis=0)
    Wc, bc = np.asarray(Wc), np.asarray(bc)

    plan = make_plan(x, edge_index, graph_ids)
    key = (plan.N, plan.E, plan.G, plan.K0, plan.K1, plan.n_layers)
    nc = _get_compiled(key, plan)

    from concourse.bass_utils import run_bass_kernel_spmd
    in_maps = build_inputs(plan, x, W_all, b_all, Wc, bc)
    res = run_bass_kernel_spmd(nc, in_maps, core_ids=list(range(plan.NC)))
    return res.results[0]["out"].astype(np.float32)
